# revision 1
# baseline (speedup 1.0000x reference)
"""Trainium2 Bass kernel for nn_CandidateSubgraphTFModel (gnn_message_passing).

Strategy (8 NeuronCores, SPMD):
  - Data-parallel over batch B=32 -> 4 rows/core for the TCN/encode path.
  - GCN sharded: each core computes its 256-row slice of H, fuses hproj
    into its slice of the candidate matrix G = (g*hproj)^T @ H^T, then an
    HBM AllGather (bf16, 2 MB) replicates G. The per-token hproj matmul
    disappears: logits = rstd * ((z - mu) @ G) * mask, with rstd*mask
    applied as the [128,1] per-partition scale of the existing PSUM->SBUF
    output copy (free).
  - Degree vector d = clip(rowsum(A)+1,1e-6)^-1/2 and A_hat^T = A^T + I
    precomputed on host (normalization prep, like the weight transposes);
    d_j folded into the X gather, d_i folded into the m1 PSUM->SBUF copy.
  - bf16 for embed/A/conv weights/x/y1/zc/G (halves gather+collective
    traffic and SBUF); conv psum accumulation and LN stats in fp32;
    z kept f32r. Measured rel err ~4e-3 vs the 2e-2 gate.
  - LN: stats via ones-matvec on PE, transposed to column form for the
    variance/rsqrt chain ([128,4] tiles) - nothing serial per supertile.
  - Software-pipelined schedule: conv(rows 0..2) emitted before
    logits(row 0) so the AllGather and the next-row token gathers are
    hidden under conv compute; row gathers are interleaved into conv1
    groups, output DMAs alternate between the two HWDGE queues.

kernel(**inputs) takes FULL inputs (as in reference.setup_inputs()) and
returns the FULL [32, 1024, 2048] logits.
"""
import sys
sys.path.insert(0, '/opt/trn_rl_repo')
import numpy as np

import concourse.bass as bass
import concourse.bacc as bacc
import concourse.tile as tile
from concourse import mybir
from concourse.masks import make_identity

f32 = mybir.dt.float32
f32r = mybir.dt.float32r
bf16 = mybir.dt.bfloat16
i32 = mybir.dt.int32
AF = mybir.ActivationFunctionType
OP = mybir.AluOpType

LN_EPS = 1e-5


class Cfg:
    def __init__(self, V=50000, D=512, B=32, S=1024, N=2048, K=3, n_cores=8):
        self.V, self.D, self.B, self.S, self.N, self.K = V, D, B, S, N, K
        self.n_cores = n_cores
        self.B_loc = B // n_cores
        self.DC = D // 128          # feature chunks
        self.ST = min(512, S)       # supertile (tokens)
        self.NTT = self.ST // 128   # token-tiles per supertile
        self.NST = S // self.ST     # supertiles per row
        self.NJC = N // 128         # GCN j chunks
        self.SW = min(512, N)       # slice width over N
        self.NSL = N // self.SW     # slices over N
        self.TOK = self.B_loc * S   # tokens per core


def build_program(cfg, reps=1, has_c0=False):
    c = cfg
    nc = bacc.Bacc("TRN2", target_bir_lowering=False, debug=False,
                   num_devices=cfg.n_cores)

    x_in = nc.dram_tensor("x_in_loc", [c.TOK], i32, kind="ExternalInput").ap()
    mask = nc.dram_tensor("mask_loc", [c.TOK], f32, kind="ExternalInput").ap()
    sub_nodes = nc.dram_tensor("sub_nodes", [c.N], i32, kind="ExternalInput").ap()
    A_T = nc.dram_tensor("A_subT", [c.N, c.N // c.n_cores], bf16,
                         kind="ExternalInput").ap()
    embed = nc.dram_tensor("embed", [c.V, c.D], bf16, kind="ExternalInput").ap()
    c1w = nc.dram_tensor("conv1_w", [c.K, c.D, c.D], f32, kind="ExternalInput").ap()
    c1b = nc.dram_tensor("conv1_b", [c.D], f32, kind="ExternalInput").ap()
    c2w = nc.dram_tensor("conv2_w", [c.K, c.D, c.D], f32, kind="ExternalInput").ap()
    c2b = nc.dram_tensor("conv2_b", [c.D], f32, kind="ExternalInput").ap()
    gwT = nc.dram_tensor("gcn_wT", [c.D, c.D], f32, kind="ExternalInput").ap()
    gb = nc.dram_tensor("gcn_b", [c.D], f32, kind="ExternalInput").ap()
    hwg = nc.dram_tensor("hproj_wg", [c.D, c.D], f32, kind="ExternalInput").ap()
    hb = nc.dram_tensor("hproj_b", [c.D], f32, kind="ExternalInput").ap()
    d_cm = nc.dram_tensor("d_col_mat", [128, c.NJC], f32, kind="ExternalInput").ap()
    d_rv = nc.dram_tensor("d_row", [1, c.N // c.n_cores], f32,
                          kind="ExternalInput").ap()
    out = nc.dram_tensor("logits_loc", [c.TOK, c.N], f32, kind="ExternalOutput").ap()

    with tile.TileContext(nc) as tc:
        for _ in range(reps):
            build_body(tc, c, x_in, mask, sub_nodes, A_T, embed, c1w, c1b,
                       c2w, c2b, gwT, gb, hwg, hb, d_cm, d_rv, out, has_c0)
    nc.compile()
    return nc


def build_body(tc, c, x_in, mask, sub_nodes, A_T, embed, c1w, c1b, c2w, c2b,
               gwT, gb, hwg, hb, d_cm, d_rv, out, has_c0=False):
    nc = tc.nc
    DC, K, ST, NTT, NST, NJC, SW, NSL = (c.DC, c.K, c.ST, c.NTT, c.NST,
                                         c.NJC, c.SW, c.NSL)

    # ---------------- pools ----------------
    const = tc.alloc_tile_pool(name="const", bufs=1)
    wp = tc.alloc_tile_pool(name="wp", bufs=1)

    # ---------------- constants ----------------
    ident = const.tile([128, 128], f32)
    make_identity(nc, ident[:])
    ones_f = const.tile([128, 1], f32)
    nc.vector.memset(ones_f[:], 1.0)
    onesr_f = const.tile([1, 128], f32)
    nc.vector.memset(onesr_f[:], 1.0)
    ones_col = const.tile([128, 1], f32r)
    nc.vector.tensor_copy(out=ones_col[:], in_=ones_f[:])
    ones_row = const.tile([1, 128], f32r)
    nc.vector.tensor_copy(out=ones_row[:], in_=onesr_f[:])
    zpad = const.tile([128, 2], f32)
    nc.vector.memset(zpad[:], 0.0)
    identb = const.tile([128, 128], bf16)
    nc.vector.tensor_copy(out=identb[:], in_=ident[:])
    zpad_b = const.tile([128, 2], bf16)
    nc.vector.memset(zpad_b[:], 0.0)

    d_col = const.tile([128, NJC], f32)
    nc.sync.dma_start(out=d_col[:], in_=d_cm[:, :])
    d_rowr = const.tile([1, c.N // c.n_cores], f32r)

    def load_cols(dram_vec, name):
        cols = []
        for dc in range(DC):
            t = const.tile([128, 1], f32, name=f"{name}_{dc}")
            nc.sync.dma_start(out=t[:], in_=dram_vec[dc*128:(dc+1)*128, None])
            cols.append(t)
        return cols

    b1_col = load_cols(c1b, "b1")
    b2_col = load_cols(c2b, "b2")
    gb_col = load_cols(gb, "gb")
    hb_col = load_cols(hb, "hb") if has_c0 else None

    # conv weights -> f32r tiles [128(din), D(dout)] per (k, din_chunk)
    stage = tc.alloc_tile_pool(name="stage", bufs=3)
    d_row_f = stage.tile([1, c.N // c.n_cores], f32, name="d_row_f", bufs=1)
    nc.sync.dma_start(out=d_row_f[:], in_=d_rv[:, :])
    nc.vector.tensor_copy(out=d_rowr[:], in_=d_row_f[:])
    # gcn weights f32r [128(d), D(e)]
    gwr = []
    for dc in range(DC):
        st_ = stage.tile([128, c.D], f32, name="wstg3")
        nc.sync.dma_start(out=st_[:], in_=gwT[dc*128:(dc+1)*128, :])
        gr = wp.tile([128, c.D], f32r, name=f"gw_{dc}")
        nc.vector.tensor_copy(out=gr[:], in_=st_[:])
        gwr.append(gr)

    # hproj*g (host-folded) f32r [128(e), D(d)]
    hwgr = []
    for ec in range(DC):
        st_ = stage.tile([128, c.D], f32, name="wstg4")
        nc.sync.dma_start(out=st_[:], in_=hwg[ec*128:(ec+1)*128, :])
        hr = wp.tile([128, c.D], f32r, name=f"hwg_{ec}")
        nc.vector.tensor_copy(out=hr[:], in_=st_[:])
        hwgr.append(hr)
    stage.release()

    # ---------------- encode SBUF (persists all rows) ----------------
    epx = tc.alloc_tile_pool(name="epx", bufs=1)
    SP = c.S + 2  # row buffer width (2 zero pad cols at left)

    x_fm_rows = {}   # row -> list of DC tiles
    m_cols_rows = {}

    def gather_piece(row, tt, psum_pool, x_fm, m_cols):
        """Gather+transpose token tile tt (of 8) of `row` into x_fm."""
        row0 = row * c.S
        t0 = tt * 128
        if tt == 0:
            for dc in range(DC):
                nc.vector.tensor_copy(out=x_fm[dc][:, 0:2], in_=zpad_b[:, :])
        idx = epx.tile([128, 1], i32, name="idx", tag="idx", bufs=8)
        nc.sync.dma_start(out=idx[:], in_=x_in[row0+t0:row0+t0+128, None])
        mc = epx.tile([128, 1], f32, name="mc", tag="mc", bufs=32)
        nc.sync.dma_start(out=mc[:], in_=mask[row0+t0:row0+t0+128, None])
        m_cols.append(mc)
        x_tm = epx.tile([128, c.D], bf16, name="x_tm", tag="x_tm", bufs=6)
        nc.gpsimd.indirect_dma_start(
            out=x_tm[:], out_offset=None, in_=embed[:],
            in_offset=bass.IndirectOffsetOnAxis(ap=idx[:, :1], axis=0))
        xm = epx.tile([128, c.D], bf16, name="xm", tag="xm", bufs=6)
        nc.vector.tensor_scalar_mul(xm[:], x_tm[:], mc[:])
        tp_ps = psum_pool.tile([128, c.D], bf16, name="tp_ps", tag="tp", bufs=1)
        for dc in range(DC):
            nc.tensor.transpose(out=tp_ps[:, dc*128:(dc+1)*128],
                                in_=xm[:, dc*128:(dc+1)*128],
                                identity=identb[:])
        for dc in range(DC):
            nc.vector.tensor_copy(out=x_fm[dc][:, 2+t0:2+t0+128],
                                  in_=tp_ps[:, dc*128:(dc+1)*128])

    def new_row_bufs(row):
        x_fm = [epx.tile([128, SP], bf16, name=f"xfm_{dcc}", tag=f"xfm{dcc}",
                         bufs=3) for dcc in range(DC)]
        x_fm_rows[row] = x_fm
        m_cols_rows[row] = []
        return x_fm, m_cols_rows[row]

    # ================= GCN phase (sharded over cores) =================
    # Each core computes its 256-row slice of H and of G, then AllGather.
    SG = c.N // c.n_cores  # 256: this core's i-slice width
    gtp = tc.alloc_tile_pool(name="gtp", bufs=1)
    G = [gtp.tile([128, c.N], bf16, name=f"G_{dc}") for dc in range(DC)]
    r0_row = gtp.tile([1, c.N], bf16, name="r0_row") if has_c0 else None

    gp = tc.alloc_tile_pool(name="gp", bufs=1)
    pgm = tc.alloc_tile_pool(name="pgm", bufs=1, space="PSUM")
    pg0 = tc.alloc_tile_pool(name="pg0", bufs=1, space="PSUM")
    dram = tc.alloc_tile_pool(name="dram", bufs=1, space="DRAM")
    NR0 = (c.D + 1) if has_c0 else c.D
    g_in = dram.tile([NR0, SG], bf16, name="g_in")
    g_out = dram.tile([c.n_cores * NR0, SG], bf16, name="g_out",
                      addr_space="Shared")

    # gather X = embed[sub_nodes]; fold d_j in the rounding copy
    Xr = []
    for jc in range(NJC):
        sidx = gp.tile([128, 1], i32, name="sidx", tag="sidx", bufs=8)
        nc.sync.dma_start(out=sidx[:], in_=sub_nodes[jc*128:(jc+1)*128, None])
        xstg = gp.tile([128, c.D], bf16, name="xstg", tag="xstg", bufs=8)
        nc.gpsimd.indirect_dma_start(
            out=xstg[:], out_offset=None, in_=embed[:],
            in_offset=bass.IndirectOffsetOnAxis(ap=sidx[:, :1], axis=0))
        xt = gp.tile([128, c.D], bf16, name=f"Xg_{jc}", tag=f"Xr{jc}")
        nc.vector.tensor_scalar_mul(xt[:], xstg[:], d_col[:, jc:jc+1])
        Xr.append(xt)

    # m1 = X~^T @ A_hatT_loc (A_hat = A + I folded on host)
    m1_ps = [pgm.tile([128, SG], f32, name=f"m1p_{dc}", tag=f"gm{dc}",
                      bufs=1) for dc in range(DC)]
    for jc in range(NJC):
        a2 = gp.tile([128, SG], bf16, name="a2", tag="a2", bufs=8)
        nc.sync.dma_start(out=a2[:], in_=A_T[jc*128:(jc+1)*128, :])
        for dc in range(DC):
            nc.tensor.matmul(out=m1_ps[dc][:],
                             lhsT=Xr[jc][:, dc*128:(dc+1)*128],
                             rhs=a2[:], start=(jc == 0),
                             stop=(jc == NJC - 1))
    # d_i broadcast for this core's slice
    db_ps = pgm.tile([128, SG], f32, name="db_ps", tag="gsc", bufs=2)
    nc.tensor.matmul(out=db_ps[:], lhsT=ones_row[:], rhs=d_rowr[0:1, :],
                     start=True, stop=True)
    db_sb = gp.tile([128, SG], f32, name="db_sb", tag="db", bufs=2)
    nc.vector.tensor_copy(out=db_sb[:], in_=db_ps[:])
    m1s = []
    for dc in range(DC):
        m1t = gp.tile([128, SG], f32r, name=f"m1s_{dc}", tag=f"m1s{dc}",
                      bufs=1)
        nc.vector.tensor_mul(m1t[:], m1_ps[dc][:], db_sb[:])
        m1s.append(m1t)
    HTs = []
    for ec in range(DC):
        h2_ps = pgm.tile([128, SG], f32, name="h2_ps", tag="gsc", bufs=2)
        for dc in range(DC):
            nc.tensor.matmul(out=h2_ps[:],
                             lhsT=gwr[dc][:, ec*128:(ec+1)*128],
                             rhs=m1s[dc][:], start=(dc == 0),
                             stop=(dc == DC - 1))
        ht = gp.tile([128, SG], f32r, name=f"HT_{ec}", tag=f"ht{ec}",
                     bufs=2)
        nc.scalar.activation(out=ht[:], in_=h2_ps[:],
                             func=AF.Relu, bias=gb_col[ec][:])
        HTs.append(ht)
    # local G slice: G_loc[dc] = sum_ec hwgr[ec][:,dc] @ HTs[ec]
    for dc in range(DC):
        g_ps = pgm.tile([128, SG], f32, name="g_ps", tag="gsc", bufs=2)
        for ec in range(DC):
            nc.tensor.matmul(out=g_ps[:],
                             lhsT=hwgr[ec][:, dc*128:(dc+1)*128],
                             rhs=HTs[ec][:], start=(ec == 0),
                             stop=(ec == DC - 1))
        gl = gp.tile([128, SG], bf16, name="gl", tag="gl", bufs=2)
        nc.vector.tensor_copy(out=gl[:], in_=g_ps[:])
        nc.sync.dma_start(out=g_in[dc*128:(dc+1)*128, :], in_=gl[:])
    if has_c0:
        r0_ps = pgm.tile([1, SG], f32, name="r0_ps", tag="gsc", bufs=2)
        for ec in range(DC):
            nc.tensor.matmul(out=r0_ps[0:1, :], lhsT=hb_col[ec][:],
                             rhs=HTs[ec][:], start=(ec == 0),
                             stop=(ec == DC - 1))
        r0l = gp.tile([1, SG], bf16, name="r0l", tag="gl", bufs=2)
        nc.vector.tensor_copy(out=r0l[:], in_=r0_ps[:])
        nc.sync.dma_start(out=g_in[c.D:c.D+1, :], in_=r0l[:])

    nc.gpsimd.collective_compute(
        "AllGather", mybir.AluOpType.bypass,
        replica_groups=[list(range(c.n_cores))],
        ins=[g_in.opt()], outs=[g_out.opt()])

    # ---------------- row 0 gather (overlaps collective) ----------------
    x_fm0, m_cols0 = new_row_bufs(0)
    for tt in range(NTT * NST):
        gather_piece(0, tt, pg0, x_fm0, m_cols0)

    gp.release()
    pg0.release()
    pgm.release()

    # conv weights (loaded after GCN so the A stream goes first)
    wcp = tc.alloc_tile_pool(name="wcp", bufs=1)
    stage2 = tc.alloc_tile_pool(name="stage2", bufs=3)
    w1r, w2r = [], []
    for (wsrc, wdst) in ((c1w, w1r), (c2w, w2r)):
        for k in range(K):
            for dc in range(DC):
                st_ = stage2.tile([128, c.D], f32, name="wstg")
                nc.sync.dma_start(out=st_[:], in_=wsrc[k, dc*128:(dc+1)*128, :])
                wr = wcp.tile([128, c.D], bf16,
                              name=f"w_{len(wdst)}_{id(wdst)%97}")
                nc.vector.tensor_copy(out=wr[:], in_=st_[:])
                wdst.append(wr)
    stage2.release()

    ep = tc.alloc_tile_pool(name="ep", bufs=1)

    # ---- load gathered G into resident tiles (both HWDGE queues) ----
    for cc in range(c.n_cores):
        for dc in range(DC):
            eng = nc.scalar if (cc * DC + dc) % 2 == 0 else nc.sync
            eng.dma_start(
                out=G[dc][:, cc*SG:(cc+1)*SG],
                in_=g_out[cc*NR0 + dc*128:cc*NR0 + (dc+1)*128, :])
        if has_c0:
            nc.scalar.dma_start(
                out=r0_row[0:1, cc*SG:(cc+1)*SG],
                in_=g_out[cc*NR0 + c.D:cc*NR0 + c.D + 1, :])

    # ================= encode + logits =================
    pe = tc.alloc_tile_pool(name="pe", bufs=1, space="PSUM")

    row_state = {}

    def conv_phase(row, gather_row=None):
        """conv1 + conv2 + stats + LN columns for `row`; optionally
        interleave gather pieces for `gather_row` into the conv1 groups."""
        x_fm = x_fm_rows[row]
        m_cols = m_cols_rows[row]
        nxt = None
        if gather_row is not None:
            nxt = new_row_bufs(gather_row)

        y1_fm = [ep.tile([128, SP], bf16, name=f"y1fm_{dcc}", tag=f"y1{dcc}",
                         bufs=1) for dcc in range(DC)]
        for dc in range(DC):
            nc.vector.tensor_copy(out=y1_fm[dc][:, 0:2], in_=zpad_b[:, :])

        piece = 0
        for dcout in range(DC):
            for st in range(NST):
                s0 = st * ST
                c1_ps = pe.tile([128, ST], f32, name="c1_ps", tag="c1", bufs=2)
                first = True
                for k in range(K):
                    for dci in range(DC):
                        nc.tensor.matmul(
                            out=c1_ps[:],
                            lhsT=w1r[k*DC+dci][:, dcout*128:(dcout+1)*128],
                            rhs=x_fm[dci][:, s0+k:s0+k+ST],
                            start=first, stop=(k == K-1 and dci == DC-1))
                        first = False
                nc.scalar.activation(out=y1_fm[dcout][:, 2+s0:2+s0+ST],
                                     in_=c1_ps[:], func=AF.Relu,
                                     bias=b1_col[dcout][:])
                if nxt is not None:
                    gather_piece(gather_row, piece, pe, nxt[0], nxt[1])
                piece += 1

        z_st, sc_st = [], []
        zzq = []
        for st in range(NST):
            s0 = st * ST
            z, zq = [], []
            for dcout in range(DC):
                c2_ps = pe.tile([128, ST], f32, name="c2_ps", tag="c2", bufs=2)
                first = True
                for k in range(K):
                    for dci in range(DC):
                        nc.tensor.matmul(
                            out=c2_ps[:],
                            lhsT=w2r[k*DC+dci][:, dcout*128:(dcout+1)*128],
                            rhs=y1_fm[dci][:, s0+k:s0+k+ST],
                            start=first, stop=(k == K-1 and dci == DC-1))
                        first = False
                zt = ep.tile([128, ST], f32r, name=f"z_{dcout}",
                             tag=f"z{dcout}", bufs=2)
                nc.vector.scalar_tensor_tensor(
                    out=zt[:], in0=c2_ps[:], scalar=b2_col[dcout][:],
                    in1=x_fm[dcout][:, 2+s0:2+s0+ST],
                    op0=OP.add, op1=OP.add)
                z.append(zt)
                zqt = ep.tile([128, ST], f32r, name="zsq", tag="zsq", bufs=8)
                nc.scalar.square(zqt[:], zt[:].bitcast(f32))
                zq.append(zqt)
            zzq.append((z, zq))
        for st in range(NST):
            s0 = st * ST
            z, zq = zzq[st]
            st_ps = pe.tile([1, ST], f32, name="st_ps", tag="sa", bufs=1)
            for dc in range(DC):
                nc.tensor.matmul(out=st_ps[0:1, :], lhsT=ones_col[:],
                                 rhs=z[dc][:], start=(dc == 0),
                                 stop=(dc == DC-1))
            sq_ps = pe.tile([1, ST], f32, name="sq_ps", tag="c2", bufs=2)
            for dc in range(DC):
                nc.tensor.matmul(out=sq_ps[0:1, :], lhsT=ones_col[:],
                                 rhs=zq[dc][:], start=(dc == 0),
                                 stop=(dc == DC-1))
            mu_row = ep.tile([1, ST], f32, name="mu_row", tag="mu_row",
                             bufs=2)
            nc.scalar.mul(mu_row[:], st_ps[0:1, :], 1.0 / c.D)
            mu_rowr = ep.tile([1, ST], f32r, name="mu_rowr", tag="mu_rowr",
                              bufs=2)
            nc.vector.tensor_copy(out=mu_rowr[:], in_=mu_row[:])
            ms_row = ep.tile([1, ST], f32, name="ms_row", tag="ms_row",
                             bufs=2)
            nc.scalar.mul(ms_row[:], sq_ps[0:1, :], 1.0 / c.D)
            tr_ps = pe.tile([128, 2*NTT], f32, name="tr_ps", tag="c2",
                            bufs=2)
            for tt in range(NTT):
                nc.tensor.transpose(out=tr_ps[:, tt:tt+1],
                                    in_=mu_row[0:1, tt*128:(tt+1)*128],
                                    identity=ident[0:1, 0:1])
            for tt in range(NTT):
                nc.tensor.transpose(out=tr_ps[:, NTT+tt:NTT+tt+1],
                                    in_=ms_row[0:1, tt*128:(tt+1)*128],
                                    identity=ident[0:1, 0:1])
            mu_bc = pe.tile([128, ST], f32, name="mu_bc", tag="c2", bufs=2)
            nc.tensor.matmul(out=mu_bc[:], lhsT=ones_row[:], rhs=mu_rowr[:],
                             start=True, stop=True)
            zc = []
            for dc in range(DC):
                zct = ep.tile([128, ST], bf16, name=f"zc_{dc}", tag=f"zc{dc}",
                              bufs=6)
                nc.vector.scalar_tensor_tensor(
                    out=zct[:], in0=mu_bc[:], scalar=-1.0,
                    in1=z[dc][:].bitcast(f32), op0=OP.mult, op1=OP.add)
                zc.append(zct)
            musq = ep.tile([128, NTT], f32, name="musq", tag="musq", bufs=2)
            nc.scalar.square(musq[:], tr_ps[:, 0:NTT])
            var_c = ep.tile([128, NTT], f32, name="var_c", tag="var_c", bufs=2)
            nc.vector.tensor_tensor(out=var_c[:], in0=tr_ps[:, NTT:2*NTT],
                                    in1=musq[:], op=OP.subtract)
            nc.vector.tensor_scalar_add(var_c[:], var_c[:], LN_EPS)
            nc.vector.reciprocal(var_c[:], var_c[:])
            rstd_c = ep.tile([128, NTT], f32, name="rstd_c", tag="rstd_c",
                             bufs=2)
            nc.scalar.sqrt(rstd_c[:], var_c[:])
            sc_cols = []
            for tt in range(NTT):
                sc = ep.tile([128, 1], f32, name="sc", tag="sc", bufs=32)
                nc.vector.tensor_scalar_mul(sc[:], rstd_c[:, tt:tt+1],
                                            m_cols[st*NTT+tt][:])
                sc_cols.append(sc)
            std_rows = None
            if has_c0:
                std_c = ep.tile([128, NTT], f32, name="std_c", tag="std_c",
                                bufs=2)
                nc.vector.tensor_tensor(out=std_c[:], in0=rstd_c[:],
                                        in1=var_c[:], op=OP.divide)
                std_rows = []
                for tt in range(NTT):
                    sr_ps = pe.tile([1, 128], f32, name="sr_ps", tag="sa",
                                    bufs=1)
                    nc.tensor.transpose(out=sr_ps[:],
                                        in_=std_c[:, tt:tt+1],
                                        identity=ident[0:1, 0:1])
                    sr = ep.tile([1, 128], bf16, name="sr", tag="sr", bufs=8)
                    nc.vector.tensor_copy(out=sr[:], in_=sr_ps[:])
                    std_rows.append(sr)
            z_st.append(zc)
            sc_st.append((sc_cols, std_rows))
        row_state[row] = (z_st, sc_st)

    def logits_phase(row, gather_row=None):
        z_st, sc_st = row_state.pop(row)
        row0 = row * c.S
        nxt = None
        if gather_row is not None:
            nxt = new_row_bufs(gather_row)
        piece = 0
        for st in range(NST):
            s0 = st * ST
            zc = z_st[st]
            sc_cols, std_rows = sc_st[st]
            for tt in range(NTT):
                for ns in range(NSL):
                    lg_ps = pe.tile([128, SW], f32, name="lg_ps", tag="lg",
                                    bufs=2)
                    for ec in range(DC):
                        nc.tensor.matmul(
                            out=lg_ps[:],
                            lhsT=zc[ec][:, tt*128:(tt+1)*128],
                            rhs=G[ec][:, ns*SW:(ns+1)*SW],
                            start=(ec == 0),
                            stop=(ec == DC-1 and not has_c0))
                        if has_c0 and ec == DC - 1:
                            nc.tensor.matmul(
                                out=lg_ps[:], lhsT=std_rows[tt][:],
                                rhs=r0_row[0:1, ns*SW:(ns+1)*SW],
                                start=False, stop=True)
                    lo = ep.tile([128, SW], f32, name="lo", tag="lo", bufs=4)
                    if ns % 2 == 0:
                        nc.scalar.mul(lo[:], lg_ps[:], sc_cols[tt][:])
                    else:
                        nc.vector.tensor_scalar_mul(lo[:], lg_ps[:],
                                                    sc_cols[tt][:])
                    t0g = row0 + s0 + tt * 128
                    eng = (nc.scalar if (ns % 2 == 0 or row == c.B_loc - 1)
                           else nc.sync)
                    eng.dma_start(out=out[t0g:t0g+128, ns*SW:(ns+1)*SW],
                                  in_=lo[:])
                if nxt is not None:
                    gather_piece(gather_row, piece, pe, nxt[0], nxt[1])
                piece += 1

    # software-pipelined schedule: three conv phases run before the first
    # logits phase so the G AllGather and next-row gathers are fully hidden
    conv_phase(0, gather_row=1)
    conv_phase(1, gather_row=2)
    conv_phase(2, gather_row=3)
    logits_phase(0)
    conv_phase(3)
    logits_phase(1)
    logits_phase(2)
    logits_phase(3)
    pe.release()
    ep.release()
    wcp.release()
    gtp.release()
    epx.release()
    wp.release()
    const.release()


# ---------------------------------------------------------------------------
# host side
# ---------------------------------------------------------------------------

_CACHE = {}


def _get_program(cfg, has_c0=False):
    key = (cfg.V, cfg.D, cfg.B, cfg.S, cfg.N, cfg.K, cfg.n_cores, has_c0)
    if key not in _CACHE:
        _CACHE[key] = build_program(cfg, has_c0=has_c0)
    return _CACHE[key]


class _Runner:
    """Direct PJRT execution (no donation) so repeated runs are cheap."""

    def __init__(self, nc, n_cores):
        import jax
        from jax.sharding import Mesh, PartitionSpec, NamedSharding
        from jax.experimental.shard_map import shard_map
        from concourse import bass2jax
        bass2jax.install_neuronx_cc_hook()
        self.jax = jax
        self.n_cores = n_cores
        part_name = nc.partition_id_tensor.name if nc.partition_id_tensor else None
        in_names, out_names, out_avals, zero_outs = [], [], [], []
        for alloc in nc.m.functions[0].allocations:
            if not isinstance(alloc, mybir.MemoryLocationSet):
                continue
            name = alloc.memorylocations[0].name
            if alloc.kind == "ExternalInput":
                if name != part_name:
                    in_names.append(name)
            elif alloc.kind == "ExternalOutput":
                out_names.append(name)
                shape = tuple(alloc.tensor_shape)
                dtype = mybir.dt.np(alloc.dtype)
                out_avals.append(jax.core.ShapedArray(shape, dtype))
                zero_outs.append(np.zeros(shape, dtype))
        self.in_names, self.out_names = in_names, out_names
        self.out_avals, self.zero_outs = out_avals, zero_outs
        self.n_params = len(in_names)
        all_in = list(in_names) + list(out_names)
        if part_name:
            all_in.append(part_name)
        out_avals_t, all_in_t, out_names_t = (tuple(out_avals), tuple(all_in),
                                              tuple(out_names))

        def _body(*args):
            operands = list(args)
            if part_name:
                operands.append(bass2jax.partition_id_tensor())
            return tuple(bass2jax._bass_exec_p.bind(
                *operands, out_avals=out_avals_t, in_names=all_in_t,
                out_names=out_names_t, lowering_input_output_aliases=(),
                sim_require_finite=True, sim_require_nnan=True, nc=nc))

        devices = jax.devices()[:n_cores]
        self.mesh = Mesh(np.asarray(devices), ("core",))
        n_io = self.n_params + len(out_names)
        self.sharded = jax.jit(
            shard_map(_body, mesh=self.mesh,
                      in_specs=(PartitionSpec("core"),) * n_io,
                      out_specs=(PartitionSpec("core"),) * len(out_names),
                      check_rep=False),
            keep_unused=True)
        self.shard = NamedSharding(self.mesh, PartitionSpec("core"))

    def set_inputs(self, in_maps):
        jax = self.jax
        per_core = [[np.asarray(m[n]) for n in self.in_names] for m in in_maps]
        concat = [np.concatenate([per_core[cc][i] for cc in range(self.n_cores)],
                                 axis=0) for i in range(self.n_params)]
        concat += [np.zeros((self.n_cores * z.shape[0], *z.shape[1:]), z.dtype)
                   for z in self.zero_outs]
        self.dev_in = [jax.device_put(a, self.shard) for a in concat]
        jax.block_until_ready(self.dev_in)

    def run(self):
        outs = self.sharded(*self.dev_in)
        self.jax.block_until_ready(outs)
        return outs

    def run_np(self):
        outs = self.run()
        return [
            {n: np.asarray(outs[i]).reshape(self.n_cores,
                                            *self.out_avals[i].shape)[cc]
             for i, n in enumerate(self.out_names)}
            for cc in range(self.n_cores)
        ]


_RUNNER = {}


def make_in_maps(cfg, inputs):
    c = cfg
    x_in = np.asarray(inputs['x_in'])
    mask = np.asarray(inputs['mask_in']).astype(np.float32)
    import ml_dtypes
    A = np.asarray(inputs['A_sub']).astype(np.float32)
    A_hatT = (A.T + np.eye(c.N, dtype=np.float32)).astype(ml_dtypes.bfloat16)
    ln_g = np.asarray(inputs['ln_g']).astype(np.float32)
    ln_b = np.asarray(inputs['ln_b']).astype(np.float32)
    hproj_w = np.asarray(inputs['hproj_w']).astype(np.float32)
    gcn_wT = np.ascontiguousarray(np.asarray(inputs['gcn_w']).T)
    # degree vector d = clip(rowsum(A)+1, 1e-6)^-0.5 (normalization prep)
    d = np.clip(A.sum(axis=1) + 1.0, 1e-6, None) ** -0.5
    d = d.astype(np.float32)
    d_col_mat = np.ascontiguousarray(d.reshape(c.NJC, 128).T)
    hproj_wg = np.ascontiguousarray(hproj_w * ln_g[None, :])
    hproj_b = np.ascontiguousarray(hproj_w @ ln_b)
    shared = {
        'sub_nodes': np.asarray(inputs['sub_nodes']).astype(np.int32),
        'embed': np.asarray(inputs['embed']).astype(ml_dtypes.bfloat16),
        'conv1_w': np.asarray(inputs['conv1_w']).astype(np.float32),
        'conv1_b': np.asarray(inputs['conv1_b']).astype(np.float32),
        'conv2_w': np.asarray(inputs['conv2_w']).astype(np.float32),
        'conv2_b': np.asarray(inputs['conv2_b']).astype(np.float32),
        'gcn_wT': gcn_wT.astype(np.float32),
        'gcn_b': np.asarray(inputs['gcn_b']).astype(np.float32),
        'hproj_wg': hproj_wg,
        'hproj_b': hproj_b.astype(np.float32),
        'd_col_mat': d_col_mat,
    }
    in_maps = []
    SG = c.N // c.n_cores
    for cc in range(c.n_cores):
        rows = slice(cc * c.B_loc, (cc + 1) * c.B_loc)
        m = dict(shared)
        m['x_in_loc'] = np.ascontiguousarray(
            x_in[rows].reshape(-1)).astype(np.int32)
        m['mask_loc'] = np.ascontiguousarray(mask[rows].reshape(-1))
        m['A_subT'] = np.ascontiguousarray(A_hatT[:, cc*SG:(cc+1)*SG])
        m['d_row'] = np.ascontiguousarray(d[None, cc*SG:(cc+1)*SG])
        in_maps.append(m)
    return in_maps


def kernel(**inputs):
    cfg = Cfg()
    has_c0 = bool(np.any(np.asarray(inputs['ln_b']) != 0))
    nc = _get_program(cfg, has_c0)
    key = id(nc)
    if key not in _RUNNER:
        _RUNNER[key] = _Runner(nc, cfg.n_cores)
    r = _RUNNER[key]
    r.set_inputs(make_in_maps(cfg, inputs))
    res = r.run_np()
    out = np.concatenate(
        [res[cc]['logits_loc'].reshape(cfg.B_loc, cfg.S, cfg.N)
         for cc in range(cfg.n_cores)], axis=0)
    return out



# revision 3
# speedup vs baseline: 1.1850x; 1.1850x over previous
"""Trainium2 Bass kernel for nn_CandidateSubgraphTFModel (gnn_message_passing).

Strategy (8 NeuronCores, SPMD):
  - Data-parallel over batch B=32 -> 4 rows/core for the TCN/encode path.
  - GCN sharded: each core computes its 256-row slice of H, fuses hproj
    into its slice of the candidate matrix G = (g*hproj)^T @ H^T, then an
    HBM AllGather (bf16, 2 MB) replicates G. The per-token hproj matmul
    disappears: logits = rstd * ((z - mu) @ G) * mask, with rstd*mask
    applied as the [128,1] per-partition scale of the PSUM->SBUF output
    copy (free).
  - Both degree scalings folded into A on host: A2T[j,i] = d_j A_hat[i,j]
    d_i, so m1 = X^T @ A2T_loc directly (no on-device normalization).
  - All weights shipped pre-cast bf16 (no f32 staging); logits output
    bf16 (host upcasts) halving the 33 MB/core output traffic.
  - Single PSUM pool shared by every phase (gacc x4 / acc x3 / sa x1
    banks) so the GCN, token gathers and conv pipeline co-schedule; the
    rep starts with row-0 token gathers so the PE never idles long
    enough to drop out of the HAM fast-clock state.
  - Software-pipelined schedule: conv(rows 0..2) emitted before
    logits(row 0) so the AllGather and next-row gathers hide under conv
    compute; row gathers interleave into conv1 groups; output DMAs
    alternate between the two HWDGE queues.

kernel(**inputs) takes FULL inputs (as in reference.setup_inputs()) and
returns the FULL [32, 1024, 2048] logits (float32).
"""
import sys
sys.path.insert(0, '/opt/trn_rl_repo')
import numpy as np

import concourse.bass as bass
import concourse.bacc as bacc
import concourse.tile as tile
from concourse import mybir
from concourse.masks import make_identity

f32 = mybir.dt.float32
f32r = mybir.dt.float32r
bf16 = mybir.dt.bfloat16
i32 = mybir.dt.int32
AF = mybir.ActivationFunctionType
OP = mybir.AluOpType

LN_EPS = 1e-5


class Cfg:
    def __init__(self, V=50000, D=512, B=32, S=1024, N=2048, K=3, n_cores=8):
        self.V, self.D, self.B, self.S, self.N, self.K = V, D, B, S, N, K
        self.n_cores = n_cores
        self.B_loc = B // n_cores
        self.DC = D // 128          # feature chunks
        self.ST = min(512, S)       # supertile (tokens)
        self.NTT = self.ST // 128   # token-tiles per supertile
        self.NST = S // self.ST     # supertiles per row
        self.NJC = N // 128         # GCN j chunks
        self.SW = min(512, N)       # slice width over N
        self.NSL = N // self.SW     # slices over N
        self.TOK = self.B_loc * S   # tokens per core


def build_program(cfg, reps=1, has_c0=False, salt=0):
    c = cfg
    nc = bacc.Bacc("TRN2", target_bir_lowering=False, debug=False,
                   num_devices=cfg.n_cores)

    x_in = nc.dram_tensor("x_in_loc", [c.TOK], i32, kind="ExternalInput").ap()
    mask = nc.dram_tensor("mask_loc", [c.TOK], f32, kind="ExternalInput").ap()
    sub_nodes = nc.dram_tensor("sub_nodes", [c.N], i32, kind="ExternalInput").ap()
    A_T = nc.dram_tensor("A_subT", [c.N, c.N // c.n_cores], bf16,
                         kind="ExternalInput").ap()
    embed = nc.dram_tensor("embed", [c.V, c.D], bf16, kind="ExternalInput").ap()
    c1w = nc.dram_tensor("conv1_w", [c.K, c.D, c.D], bf16, kind="ExternalInput").ap()
    c1b = nc.dram_tensor("conv1_b", [c.D], f32, kind="ExternalInput").ap()
    c2w = nc.dram_tensor("conv2_w", [c.K, c.D, c.D], bf16, kind="ExternalInput").ap()
    c2b = nc.dram_tensor("conv2_b", [c.D], f32, kind="ExternalInput").ap()
    gwT = nc.dram_tensor("gcn_wT", [c.D, c.D], bf16, kind="ExternalInput").ap()
    gb = nc.dram_tensor("gcn_b", [c.D], f32, kind="ExternalInput").ap()
    hwg = nc.dram_tensor("hproj_wg", [c.D, c.D], bf16, kind="ExternalInput").ap()
    hb = nc.dram_tensor("hproj_b", [c.D], f32, kind="ExternalInput").ap()
    out = nc.dram_tensor("logits_loc", [c.TOK, c.N], bf16,
                         kind="ExternalOutput").ap()

    with tile.TileContext(nc) as tc:
        for _ in range(reps):
            build_body(tc, c, x_in, mask, sub_nodes, A_T, embed, c1w, c1b,
                       c2w, c2b, gwT, gb, hwg, hb, out, has_c0, salt)
    nc.compile()
    return nc


def build_body(tc, c, x_in, mask, sub_nodes, A_T, embed, c1w, c1b, c2w, c2b,
               gwT, gb, hwg, hb, out, has_c0=False, salt=0):
    nc = tc.nc
    DC, K, ST, NTT, NST, NJC, SW, NSL = (c.DC, c.K, c.ST, c.NTT, c.NST,
                                         c.NJC, c.SW, c.NSL)
    SG = c.N // c.n_cores  # this core's GCN i-slice width

    # ---------------- pools ----------------
    const = tc.alloc_tile_pool(name=f"const{salt}", bufs=1)
    wp = tc.alloc_tile_pool(name="wp", bufs=1)
    gtp = tc.alloc_tile_pool(name="gtp", bufs=1)   # resident G
    epx = tc.alloc_tile_pool(name="epx", bufs=1)   # row gather bufs
    gp = tc.alloc_tile_pool(name="gp", bufs=1)     # GCN sbuf temps
    ep = tc.alloc_tile_pool(name="ep", bufs=1)     # conv temps
    pe = tc.alloc_tile_pool(name="pe", bufs=1, space="PSUM")
    dram = tc.alloc_tile_pool(name="dram", bufs=1, space="DRAM")

    # ---------------- constants ----------------
    ident = const.tile([128, 128], f32)
    make_identity(nc, ident[:])
    ones_f = const.tile([128, 1], f32)
    nc.vector.memset(ones_f[:], 1.0)
    onesr_f = const.tile([1, 128], f32)
    nc.vector.memset(onesr_f[:], 1.0)
    ones_col = const.tile([128, 1], f32r)
    nc.vector.tensor_copy(out=ones_col[:], in_=ones_f[:])
    ones_row = const.tile([1, 128], f32r)
    nc.vector.tensor_copy(out=ones_row[:], in_=onesr_f[:])
    identb = const.tile([128, 128], bf16)
    nc.vector.tensor_copy(out=identb[:], in_=ident[:])
    zpad_b = const.tile([128, 2], bf16)
    nc.vector.memset(zpad_b[:], 0.0)

    def load_cols(dram_vec, name):
        cols = []
        for dc in range(DC):
            t = const.tile([128, 1], f32, name=f"{name}_{dc}")
            nc.scalar.dma_start(out=t[:], in_=dram_vec[dc*128:(dc+1)*128, None])
            cols.append(t)
        return cols

    b1_col = load_cols(c1b, "b1")
    b2_col = load_cols(c2b, "b2")
    gb_col = load_cols(gb, "gb")
    hb_col = load_cols(hb, "hb") if has_c0 else None

    # ---- conv weights: direct bf16 loads [128(din), D(dout)] per (k, dc)
    w1r, w2r = [], []
    for (wsrc, wdst, nm) in ((c1w, w1r, "w1"), (c2w, w2r, "w2")):
        for k in range(K):
            for dc in range(DC):
                wr = wp.tile([128, c.D], bf16, name=f"{nm}_{k}_{dc}")
                nc.sync.dma_start(out=wr[:], in_=wsrc[k, dc*128:(dc+1)*128, :])
                wdst.append(wr)
    # gcn weights bf16 [128(d), D(e)] ; hproj*g bf16 [128(e), D(d)]
    gwr, hwgr = [], []
    for dc in range(DC):
        gr = wp.tile([128, c.D], bf16, name=f"gw_{dc}")
        nc.scalar.dma_start(out=gr[:], in_=gwT[dc*128:(dc+1)*128, :])
        gwr.append(gr)
    for ec in range(DC):
        hr = wp.tile([128, c.D], bf16, name=f"hwg_{ec}")
        nc.scalar.dma_start(out=hr[:], in_=hwg[ec*128:(ec+1)*128, :])
        hwgr.append(hr)

    # ---------------- encode row buffers ----------------
    SP = c.S + 2  # row buffer width (2 zero pad cols at left)
    NPC = NTT * NST  # gather pieces (token tiles) per row

    x_fm_rows = {}
    m_rows = {}

    def new_row_bufs(row):
        x_fm = [epx.tile([128, SP], bf16, name=f"xfm_{dcc}", tag=f"xfm{dcc}",
                         bufs=3) for dcc in range(DC)]
        idxr = epx.tile([128, NPC], i32, name="idxr", tag="idxr", bufs=3)
        nc.sync.dma_start(
            out=idxr[:], in_=x_in.tensor.ap()[row*c.S:(row+1)*c.S]
            .rearrange("(t p) -> p t", p=128))
        mr = epx.tile([128, NPC], f32, name="mr", tag="mr", bufs=4)
        nc.sync.dma_start(
            out=mr[:], in_=mask.tensor.ap()[row*c.S:(row+1)*c.S]
            .rearrange("(t p) -> p t", p=128))
        x_fm_rows[row] = x_fm
        m_rows[row] = mr
        return x_fm, idxr, mr

    def gather_piece(row, tt, x_fm, idxr):
        """Gather+transpose token tile tt (of NPC) of `row` into x_fm."""
        t0 = tt * 128
        if tt == 0:
            for dc in range(DC):
                nc.vector.tensor_copy(out=x_fm[dc][:, 0:2], in_=zpad_b[:, :])
        x_tm = epx.tile([128, c.D], bf16, name="x_tm", tag="x_tm", bufs=6)
        nc.gpsimd.indirect_dma_start(
            out=x_tm[:], out_offset=None, in_=embed[:],
            in_offset=bass.IndirectOffsetOnAxis(ap=idxr[:, tt:tt+1], axis=0))
        xm = epx.tile([128, c.D], bf16, name="xm", tag="xm", bufs=6)
        nc.vector.tensor_scalar_mul(xm[:], x_tm[:], m_rows[row][:, tt:tt+1])
        tp_ps = pe.tile([128, c.D], bf16, name="tp_ps", tag="acc", bufs=3)
        for dc in range(DC):
            nc.tensor.transpose(out=tp_ps[:, dc*128:(dc+1)*128],
                                in_=xm[:, dc*128:(dc+1)*128],
                                identity=identb[:])
        for dc in range(DC):
            nc.vector.tensor_copy(out=x_fm[dc][:, 2+t0:2+t0+128],
                                  in_=tp_ps[:, dc*128:(dc+1)*128])

    # ================= row 0 gather first (keeps PE warm at rep start) ==
    x_fm0, idxr0, _ = new_row_bufs(0)
    for tt in range(NPC):
        gather_piece(0, tt, x_fm0, idxr0)

    # ================= GCN (sharded over cores) =================
    # gather X = embed[sub_nodes]
    sidxr = gp.tile([128, NJC], i32, name="sidxr")
    nc.sync.dma_start(out=sidxr[:],
                      in_=sub_nodes.tensor.ap().rearrange("(t p) -> p t", p=128))
    Xr = []
    for jc in range(NJC):
        xt = gp.tile([128, c.D], bf16, name=f"Xg_{jc}", tag=f"Xr{jc}")
        nc.gpsimd.indirect_dma_start(
            out=xt[:], out_offset=None, in_=embed[:],
            in_offset=bass.IndirectOffsetOnAxis(ap=sidxr[:, jc:jc+1], axis=0))
        Xr.append(xt)

    # m1 = X^T @ A2T_loc (A2 = d_i A_hat d_j folded on host)
    m1_ps = [pe.tile([128, 512], f32, name=f"m1p_{dc}", tag="gacc",
                     bufs=4) for dc in range(DC)]
    for jc in range(NJC):
        a2 = gp.tile([128, SG], bf16, name="a2", tag="a2", bufs=8)
        nc.sync.dma_start(out=a2[:], in_=A_T[jc*128:(jc+1)*128, :])
        for dc in range(DC):
            nc.tensor.matmul(out=m1_ps[dc][:, :SG],
                             lhsT=Xr[jc][:, dc*128:(dc+1)*128],
                             rhs=a2[:], start=(jc == 0),
                             stop=(jc == NJC - 1))
    m1s = []
    for dc in range(DC):
        m1t = gp.tile([128, SG], bf16, name=f"m1s_{dc}", tag=f"m1s{dc}")
        nc.vector.tensor_copy(out=m1t[:], in_=m1_ps[dc][:, :SG])
        m1s.append(m1t)
    HTs = []
    for ec in range(DC):
        h2_ps = pe.tile([128, 512], f32, name="h2_ps", tag="gacc", bufs=4)
        for dc in range(DC):
            nc.tensor.matmul(out=h2_ps[:, :SG],
                             lhsT=gwr[dc][:, ec*128:(ec+1)*128],
                             rhs=m1s[dc][:], start=(dc == 0),
                             stop=(dc == DC - 1))
        ht = gp.tile([128, SG], bf16, name=f"HT_{ec}", tag=f"ht{ec}")
        nc.scalar.activation(out=ht[:], in_=h2_ps[:, :SG],
                             func=AF.Relu, bias=gb_col[ec][:])
        HTs.append(ht)
    # local G slice: G_loc[dc] = sum_ec hwgr[ec][:,dc] @ HTs[ec]
    NR0 = (c.D + 1) if has_c0 else c.D
    g_in = dram.tile([NR0, SG], bf16, name="g_in")
    g_out = dram.tile([c.n_cores * NR0, SG], bf16, name="g_out",
                      addr_space="Shared")
    for dc in range(DC):
        g_ps = pe.tile([128, 512], f32, name="g_ps", tag="gacc", bufs=4)
        for ec in range(DC):
            nc.tensor.matmul(out=g_ps[:, :SG],
                             lhsT=hwgr[ec][:, dc*128:(dc+1)*128],
                             rhs=HTs[ec][:], start=(ec == 0),
                             stop=(ec == DC - 1))
        gl = gp.tile([128, SG], bf16, name="gl", tag="gl", bufs=2)
        nc.vector.tensor_copy(out=gl[:], in_=g_ps[:, :SG])
        nc.sync.dma_start(out=g_in[dc*128:(dc+1)*128, :], in_=gl[:])
    if has_c0:
        r0_ps = pe.tile([1, 512], f32, name="r0_ps", tag="sa", bufs=1)
        for ec in range(DC):
            nc.tensor.matmul(out=r0_ps[0:1, :SG], lhsT=hb_col[ec][:],
                             rhs=HTs[ec][:], start=(ec == 0),
                             stop=(ec == DC - 1))
        r0l = gp.tile([1, SG], bf16, name="r0l", tag="gl", bufs=2)
        nc.vector.tensor_copy(out=r0l[:], in_=r0_ps[0:1, :SG])
        nc.sync.dma_start(out=g_in[c.D:c.D+1, :], in_=r0l[:])

    nc.gpsimd.collective_compute(
        "AllGather", mybir.AluOpType.bypass,
        replica_groups=[list(range(c.n_cores))],
        ins=[g_in.opt()], outs=[g_out.opt()])

    # ---- load gathered G into resident tiles (both HWDGE queues) ----
    G = [gtp.tile([128, c.N], bf16, name=f"G_{dc}") for dc in range(DC)]
    r0_row = gtp.tile([1, c.N], bf16, name="r0_row") if has_c0 else None
    for cc in range(c.n_cores):
        for dc in range(DC):
            eng = nc.scalar if (cc * DC + dc) % 2 == 0 else nc.sync
            eng.dma_start(
                out=G[dc][:, cc*SG:(cc+1)*SG],
                in_=g_out[cc*NR0 + dc*128:cc*NR0 + (dc+1)*128, :])
        if has_c0:
            nc.scalar.dma_start(
                out=r0_row[0:1, cc*SG:(cc+1)*SG],
                in_=g_out[cc*NR0 + c.D:cc*NR0 + c.D + 1, :])

    # ================= encode + logits =================
    row_state = {}

    def conv_phase(row, gather_row=None):
        """conv1 + conv2 + stats + LN columns for `row`; optionally
        interleave gather pieces for `gather_row` into the conv1 groups."""
        x_fm = x_fm_rows[row]
        mr = m_rows[row]
        nxt = None
        if gather_row is not None:
            nxt = new_row_bufs(gather_row)

        y1_fm = [ep.tile([128, SP], bf16, name=f"y1fm_{dcc}", tag=f"y1{dcc}",
                         bufs=1) for dcc in range(DC)]
        for dc in range(DC):
            nc.vector.tensor_copy(out=y1_fm[dc][:, 0:2], in_=zpad_b[:, :])

        piece = 0
        for dcout in range(DC):
            for st in range(NST):
                s0 = st * ST
                c1_ps = pe.tile([128, ST], f32, name="c1_ps", tag="acc",
                                bufs=3)
                first = True
                for k in range(K):
                    for dci in range(DC):
                        nc.tensor.matmul(
                            out=c1_ps[:],
                            lhsT=w1r[k*DC+dci][:, dcout*128:(dcout+1)*128],
                            rhs=x_fm[dci][:, s0+k:s0+k+ST],
                            start=first, stop=(k == K-1 and dci == DC-1))
                        first = False
                nc.scalar.activation(out=y1_fm[dcout][:, 2+s0:2+s0+ST],
                                     in_=c1_ps[:], func=AF.Relu,
                                     bias=b1_col[dcout][:])
                if nxt is not None:
                    gather_piece(gather_row, piece, nxt[0], nxt[1])
                piece += 1

        z_st, sc_st = [], []
        zzq = []
        for st in range(NST):
            s0 = st * ST
            z, zq = [], []
            for dcout in range(DC):
                c2_ps = pe.tile([128, ST], f32, name="c2_ps", tag="acc",
                                bufs=3)
                first = True
                for k in range(K):
                    for dci in range(DC):
                        nc.tensor.matmul(
                            out=c2_ps[:],
                            lhsT=w2r[k*DC+dci][:, dcout*128:(dcout+1)*128],
                            rhs=y1_fm[dci][:, s0+k:s0+k+ST],
                            start=first, stop=(k == K-1 and dci == DC-1))
                        first = False
                zt = ep.tile([128, ST], f32r, name=f"z_{dcout}",
                             tag=f"z{dcout}", bufs=2)
                nc.vector.scalar_tensor_tensor(
                    out=zt[:], in0=c2_ps[:], scalar=b2_col[dcout][:],
                    in1=x_fm[dcout][:, 2+s0:2+s0+ST],
                    op0=OP.add, op1=OP.add)
                z.append(zt)
                zqt = ep.tile([128, ST], f32r, name="zsq", tag="zsq", bufs=8)
                nc.scalar.square(zqt[:], zt[:].bitcast(f32))
                zq.append(zqt)
            zzq.append((z, zq))
        for st in range(NST):
            s0 = st * ST
            z, zq = zzq[st]
            st_ps = pe.tile([1, ST], f32, name="st_ps", tag="sa", bufs=1)
            for dc in range(DC):
                nc.tensor.matmul(out=st_ps[0:1, :], lhsT=ones_col[:],
                                 rhs=z[dc][:], start=(dc == 0),
                                 stop=(dc == DC-1))
            sq_ps = pe.tile([1, ST], f32, name="sq_ps", tag="acc", bufs=3)
            for dc in range(DC):
                nc.tensor.matmul(out=sq_ps[0:1, :], lhsT=ones_col[:],
                                 rhs=zq[dc][:], start=(dc == 0),
                                 stop=(dc == DC-1))
            mu_row = ep.tile([1, ST], f32, name="mu_row", tag="mu_row",
                             bufs=2)
            nc.scalar.mul(mu_row[:], st_ps[0:1, :], 1.0 / c.D)
            mu_rowr = ep.tile([1, ST], f32r, name="mu_rowr", tag="mu_rowr",
                              bufs=2)
            nc.vector.tensor_copy(out=mu_rowr[:], in_=mu_row[:])
            ms_row = ep.tile([1, ST], f32, name="ms_row", tag="ms_row",
                             bufs=2)
            nc.scalar.mul(ms_row[:], sq_ps[0:1, :], 1.0 / c.D)
            tr_ps = pe.tile([128, 2*NTT], f32, name="tr_ps", tag="acc",
                            bufs=3)
            for tt in range(NTT):
                nc.tensor.transpose(out=tr_ps[:, tt:tt+1],
                                    in_=mu_row[0:1, tt*128:(tt+1)*128],
                                    identity=ident[0:1, 0:1])
            for tt in range(NTT):
                nc.tensor.transpose(out=tr_ps[:, NTT+tt:NTT+tt+1],
                                    in_=ms_row[0:1, tt*128:(tt+1)*128],
                                    identity=ident[0:1, 0:1])
            mu_bc = pe.tile([128, ST], f32, name="mu_bc", tag="acc", bufs=3)
            nc.tensor.matmul(out=mu_bc[:], lhsT=ones_row[:], rhs=mu_rowr[:],
                             start=True, stop=True)
            zc = []
            for dc in range(DC):
                zct = ep.tile([128, ST], bf16, name=f"zc_{dc}", tag=f"zc{dc}",
                              bufs=6)
                nc.vector.scalar_tensor_tensor(
                    out=zct[:], in0=mu_bc[:], scalar=-1.0,
                    in1=z[dc][:].bitcast(f32), op0=OP.mult, op1=OP.add)
                zc.append(zct)
            musq = ep.tile([128, NTT], f32, name="musq", tag="musq", bufs=2)
            nc.scalar.square(musq[:], tr_ps[:, 0:NTT])
            var_c = ep.tile([128, NTT], f32, name="var_c", tag="var_c", bufs=2)
            nc.vector.tensor_tensor(out=var_c[:], in0=tr_ps[:, NTT:2*NTT],
                                    in1=musq[:], op=OP.subtract)
            nc.vector.tensor_scalar_add(var_c[:], var_c[:], LN_EPS)
            nc.vector.reciprocal(var_c[:], var_c[:])
            rstd_c = ep.tile([128, NTT], f32, name="rstd_c", tag="rstd_c",
                             bufs=2)
            nc.scalar.sqrt(rstd_c[:], var_c[:])
            sc_cols = []
            for tt in range(NTT):
                sc = ep.tile([128, 1], f32, name="sc", tag="sc", bufs=32)
                nc.vector.tensor_scalar_mul(sc[:], rstd_c[:, tt:tt+1],
                                            mr[:, st*NTT+tt:st*NTT+tt+1])
                sc_cols.append(sc)
            std_rows = None
            if has_c0:
                std_c = ep.tile([128, NTT], f32, name="std_c", tag="std_c",
                                bufs=2)
                nc.vector.tensor_tensor(out=std_c[:], in0=rstd_c[:],
                                        in1=var_c[:], op=OP.divide)
                std_rows = []
                for tt in range(NTT):
                    sr_ps = pe.tile([1, 128], f32, name="sr_ps", tag="sa",
                                    bufs=1)
                    nc.tensor.transpose(out=sr_ps[:],
                                        in_=std_c[:, tt:tt+1],
                                        identity=ident[0:1, 0:1])
                    sr = ep.tile([1, 128], bf16, name="sr", tag="sr", bufs=8)
                    nc.vector.tensor_copy(out=sr[:], in_=sr_ps[:])
                    std_rows.append(sr)
            z_st.append(zc)
            sc_st.append((sc_cols, std_rows))
        row_state[row] = (z_st, sc_st)

    def logits_phase(row, gather_row=None):
        z_st, sc_st = row_state.pop(row)
        row0 = row * c.S
        nxt = None
        if gather_row is not None:
            nxt = new_row_bufs(gather_row)
        piece = 0
        for st in range(NST):
            s0 = st * ST
            zc = z_st[st]
            sc_cols, std_rows = sc_st[st]
            for tt in range(NTT):
                for ns in range(NSL):
                    lg_ps = pe.tile([128, SW], f32, name="lg_ps", tag="gacc",
                                    bufs=4)
                    for ec in range(DC):
                        nc.tensor.matmul(
                            out=lg_ps[:],
                            lhsT=zc[ec][:, tt*128:(tt+1)*128],
                            rhs=G[ec][:, ns*SW:(ns+1)*SW],
                            start=(ec == 0),
                            stop=(ec == DC-1 and not has_c0))
                        if has_c0 and ec == DC - 1:
                            nc.tensor.matmul(
                                out=lg_ps[:], lhsT=std_rows[tt][:],
                                rhs=r0_row[0:1, ns*SW:(ns+1)*SW],
                                start=False, stop=True)
                    lo = ep.tile([128, SW], bf16, name="lo", tag="lo", bufs=6)
                    if ns % 2 == 0:
                        nc.scalar.mul(lo[:], lg_ps[:], sc_cols[tt][:])
                    else:
                        nc.vector.tensor_scalar_mul(lo[:], lg_ps[:],
                                                    sc_cols[tt][:])
                    t0g = row0 + s0 + tt * 128
                    eng = (nc.scalar if (ns % 2 == 0 or row == c.B_loc - 1)
                           else nc.sync)
                    eng.dma_start(out=out[t0g:t0g+128, ns*SW:(ns+1)*SW],
                                  in_=lo[:])
                if nxt is not None:
                    gather_piece(gather_row, piece, nxt[0], nxt[1])
                piece += 1

    # software-pipelined schedule: three conv phases run before the first
    # logits phase so the G AllGather and next-row gathers are fully hidden
    conv_phase(0, gather_row=1)
    conv_phase(1, gather_row=2)
    conv_phase(2, gather_row=3)
    logits_phase(0)
    conv_phase(3)
    logits_phase(1)
    logits_phase(2)
    logits_phase(3)
    pe.release()
    ep.release()
    gp.release()
    epx.release()
    gtp.release()
    wp.release()
    const.release()


# ---------------------------------------------------------------------------
# host side
# ---------------------------------------------------------------------------

_CACHE = {}


def _get_program(cfg, has_c0=False):
    key = (cfg.V, cfg.D, cfg.B, cfg.S, cfg.N, cfg.K, cfg.n_cores, has_c0)
    if key not in _CACHE:
        err = None
        for attempt in range(4):
            try:
                _CACHE[key] = build_program(cfg, has_c0=has_c0, salt=attempt)
                err = None
                break
            except Exception as e:  # flaky scheduler deadlock -> retry
                if 'Deadlock' not in type(e).__name__ + str(e):
                    raise
                err = e
        if err is not None:
            raise err
    return _CACHE[key]


class _Runner:
    """Direct PJRT execution (no donation) so repeated runs are cheap."""

    def __init__(self, nc, n_cores):
        import jax
        from jax.sharding import Mesh, PartitionSpec, NamedSharding
        from jax.experimental.shard_map import shard_map
        from concourse import bass2jax
        bass2jax.install_neuronx_cc_hook()
        self.jax = jax
        self.n_cores = n_cores
        part_name = nc.partition_id_tensor.name if nc.partition_id_tensor else None
        in_names, out_names, out_avals, zero_outs = [], [], [], []
        for alloc in nc.m.functions[0].allocations:
            if not isinstance(alloc, mybir.MemoryLocationSet):
                continue
            name = alloc.memorylocations[0].name
            if alloc.kind == "ExternalInput":
                if name != part_name:
                    in_names.append(name)
            elif alloc.kind == "ExternalOutput":
                out_names.append(name)
                shape = tuple(alloc.tensor_shape)
                dtype = mybir.dt.np(alloc.dtype)
                out_avals.append(jax.core.ShapedArray(shape, dtype))
                zero_outs.append(np.zeros(shape, dtype))
        self.in_names, self.out_names = in_names, out_names
        self.out_avals, self.zero_outs = out_avals, zero_outs
        self.n_params = len(in_names)
        all_in = list(in_names) + list(out_names)
        if part_name:
            all_in.append(part_name)
        out_avals_t, all_in_t, out_names_t = (tuple(out_avals), tuple(all_in),
                                              tuple(out_names))

        def _body(*args):
            operands = list(args)
            if part_name:
                operands.append(bass2jax.partition_id_tensor())
            return tuple(bass2jax._bass_exec_p.bind(
                *operands, out_avals=out_avals_t, in_names=all_in_t,
                out_names=out_names_t, lowering_input_output_aliases=(),
                sim_require_finite=True, sim_require_nnan=True, nc=nc))

        devices = jax.devices()[:n_cores]
        self.mesh = Mesh(np.asarray(devices), ("core",))
        n_io = self.n_params + len(out_names)
        self.sharded = jax.jit(
            shard_map(_body, mesh=self.mesh,
                      in_specs=(PartitionSpec("core"),) * n_io,
                      out_specs=(PartitionSpec("core"),) * len(out_names),
                      check_rep=False),
            keep_unused=True)
        self.shard = NamedSharding(self.mesh, PartitionSpec("core"))

    def set_inputs(self, in_maps):
        jax = self.jax
        per_core = [[np.asarray(m[n]) for n in self.in_names] for m in in_maps]
        concat = [np.concatenate([per_core[cc][i] for cc in range(self.n_cores)],
                                 axis=0) for i in range(self.n_params)]
        concat += [np.zeros((self.n_cores * z.shape[0], *z.shape[1:]), z.dtype)
                   for z in self.zero_outs]
        self.dev_in = [jax.device_put(a, self.shard) for a in concat]
        jax.block_until_ready(self.dev_in)

    def run(self):
        outs = self.sharded(*self.dev_in)
        self.jax.block_until_ready(outs)
        return outs

    def run_np(self):
        outs = self.run()
        return [
            {n: np.asarray(outs[i]).reshape(self.n_cores,
                                            *self.out_avals[i].shape)[cc]
             for i, n in enumerate(self.out_names)}
            for cc in range(self.n_cores)
        ]


_RUNNER = {}


def make_in_maps(cfg, inputs):
    c = cfg
    import ml_dtypes
    x_in = np.asarray(inputs['x_in'])
    mask = np.asarray(inputs['mask_in']).astype(np.float32)
    A = np.asarray(inputs['A_sub']).astype(np.float32)
    # degree vector d = clip(rowsum(A)+1, 1e-6)^-0.5; fold BOTH scalings
    # into A2T[j, i] = d_j * A_hat[i, j] * d_i (normalization prep)
    d = (np.clip(A.sum(axis=1) + 1.0, 1e-6, None) ** -0.5).astype(np.float32)
    A2T = ((A.T + np.eye(c.N, dtype=np.float32))
           * d[:, None] * d[None, :]).astype(ml_dtypes.bfloat16)
    ln_g = np.asarray(inputs['ln_g']).astype(np.float32)
    ln_b = np.asarray(inputs['ln_b']).astype(np.float32)
    hproj_w = np.asarray(inputs['hproj_w']).astype(np.float32)
    gcn_wT = np.ascontiguousarray(np.asarray(inputs['gcn_w']).T)
    hproj_wg = np.ascontiguousarray(hproj_w * ln_g[None, :])
    hproj_b = np.ascontiguousarray(hproj_w @ ln_b)
    shared = {
        'sub_nodes': np.asarray(inputs['sub_nodes']).astype(np.int32),
        'embed': np.asarray(inputs['embed']).astype(ml_dtypes.bfloat16),
        'conv1_w': np.asarray(inputs['conv1_w']).astype(ml_dtypes.bfloat16),
        'conv1_b': np.asarray(inputs['conv1_b']).astype(np.float32),
        'conv2_w': np.asarray(inputs['conv2_w']).astype(ml_dtypes.bfloat16),
        'conv2_b': np.asarray(inputs['conv2_b']).astype(np.float32),
        'gcn_wT': gcn_wT.astype(ml_dtypes.bfloat16),
        'gcn_b': np.asarray(inputs['gcn_b']).astype(np.float32),
        'hproj_wg': hproj_wg.astype(ml_dtypes.bfloat16),
        'hproj_b': hproj_b.astype(np.float32),
    }
    in_maps = []
    SG = c.N // c.n_cores
    for cc in range(c.n_cores):
        rows = slice(cc * c.B_loc, (cc + 1) * c.B_loc)
        m = dict(shared)
        m['x_in_loc'] = np.ascontiguousarray(
            x_in[rows].reshape(-1)).astype(np.int32)
        m['mask_loc'] = np.ascontiguousarray(mask[rows].reshape(-1))
        m['A_subT'] = np.ascontiguousarray(A2T[:, cc*SG:(cc+1)*SG])
        in_maps.append(m)
    return in_maps


def kernel(**inputs):
    cfg = Cfg()
    has_c0 = bool(np.any(np.asarray(inputs['ln_b']) != 0))
    nc = _get_program(cfg, has_c0)
    key = id(nc)
    if key not in _RUNNER:
        _RUNNER[key] = _Runner(nc, cfg.n_cores)
    r = _RUNNER[key]
    r.set_inputs(make_in_maps(cfg, inputs))
    res = r.run_np()
    out = np.concatenate(
        [res[cc]['logits_loc'].reshape(cfg.B_loc, cfg.S, cfg.N)
         for cc in range(cfg.n_cores)], axis=0).astype(np.float32)
    return out
